# revision 1
# baseline (speedup 1.0000x reference)
"""Multi-head self-attention (B=2, N=4096, D=512, h=8, d=64) on 8 TRN2 cores.

Sharding: batch*head-pair across the 8 cores (core c -> batch c//4, heads
2*(c%4), 2*(c%4)+1). Each core computes its two heads' q/k/v projections,
flash-style attention (scores kept transposed [j, i] so no P-matrix
transposes are ever needed; softmax denominators come from a ones-augmented
V stationary), and its partial output projection. Host sums the 4 partials
per batch and adds bo. No cross-core communication.

All matmuls run in bf16 (PE stays on the warm 2.4 GHz clock; fp32r's
transpose-mode path never warms the HAM clock gate and ran 2.7x slower).
"""

import numpy as np
import ml_dtypes

import concourse.bass as bass
import concourse.tile as tile
from concourse import bacc, mybir
from concourse.bass_utils import run_bass_kernel_spmd
from concourse.masks import make_identity

F32 = mybir.dt.float32
BF16 = mybir.dt.bfloat16

B, N, D = 2, 4096, 512
HEADS, DH = 8, 64
SCALE = DH ** -0.5          # 0.125
IC = 1024                   # i-chunk (query cols per psum-out accumulation)
N_IC = N // IC              # 4
N_JC = N // 128             # 32 j-chunks (key rows per matmul = partitions)
N_CORES = 8


def build_kernel():
    nc = bacc.Bacc("TRN2", target_bir_lowering=False, debug=False)
    xT_d = nc.dram_tensor("xT", [D, N], BF16, kind="ExternalInput").ap()
    wq_d = nc.dram_tensor("wq", [D, 128], BF16, kind="ExternalInput").ap()
    wk_d = nc.dram_tensor("wk", [D, 128], BF16, kind="ExternalInput").ap()
    wv_d = nc.dram_tensor("wv", [D, 128], BF16, kind="ExternalInput").ap()
    wo_d = nc.dram_tensor("wo", [128, D], BF16, kind="ExternalInput").ap()
    pT_d = nc.dram_tensor("pT", [D, N], F32, kind="ExternalOutput").ap()
    # denominator-reciprocal scratch: one 1024-row per (ic, head)
    dd1 = nc.dram_tensor("dscr1", [8, 1024], F32).ap()
    dd2 = nc.dram_tensor("dscr2", [8, 1024], F32).ap()

    with tile.TileContext(nc) as tc:
        with (
            tc.tile_pool(name="const", bufs=1) as const_pool,
            tc.tile_pool(name="proj", bufs=1) as proj_pool,
            tc.tile_pool(name="pt", bufs=3) as pt_pool,
            tc.tile_pool(name="norm", bufs=2) as norm_pool,
            tc.tile_pool(name="stage", bufs=3) as stage_pool,
            tc.tile_pool(name="ps", bufs=2, space="PSUM") as ps_pool,
            tc.tile_pool(name="p1", bufs=2, space="PSUM") as p1_pool,
            tc.tile_pool(name="po", bufs=1, space="PSUM") as po_pool,
        ):
            # ---- P0: loads + constants -------------------------------------
            w_sb = {}
            for nm, d_ap in (("wq", wq_d), ("wk", wk_d), ("wv", wv_d)):
                t = const_pool.tile([128, 4, 128], BF16, name=f"{nm}s", tag=f"{nm}s")
                nc.sync.dma_start(t[:], d_ap.rearrange("(c p) e -> p c e", p=128))
                w_sb[nm] = t
            wo_sb = const_pool.tile([128, D], BF16, name="wos", tag="wos")
            nc.sync.dma_start(wo_sb[:], wo_d[:])
            xt_sb = []
            for dc in range(4):
                t = const_pool.tile([128, N], BF16, name=f"xt{dc}", tag=f"xt{dc}")
                xt_sb.append(t)
            for i8 in range(8):
                for dc in range(4):
                    sl = slice(i8 * 512, (i8 + 1) * 512)
                    nc.sync.dma_start(xt_sb[dc][:, sl],
                                      xT_d[dc * 128:(dc + 1) * 128, sl])
            ident_f = const_pool.tile([128, 128], F32, name="ident_f",
                                      tag="ident_f")
            make_identity(nc, ident_f[:])
            ident = const_pool.tile([128, 128], BF16, name="ident", tag="ident")
            nc.vector.tensor_copy(ident[:], ident_f[:])
            # touch Exp once so the ACT table loads during the projection phase
            escr = const_pool.tile([1, 2], F32, name="escr", tag="escr")
            nc.scalar.activation(escr[:], ident_f[0:1, 0:2],
                                 mybir.ActivationFunctionType.Exp)

            # ---- P1: projections -------------------------------------------
            # Per-head q^T/k^T with K zero-padded to 128: head h occupies
            # partitions h*64..h*64+63, the other 64 partitions are zero.
            # K=64 matmuls never warm the PE HAM clock gate (measured 463 vs
            # 219 ns at K=128 for N=512), so we pay SBUF, not cycles.
            qTh = [proj_pool.tile([128, N], BF16, name=f"qTh{h}", tag=f"qTh{h}")
                   for h in range(2)]
            kTh = [proj_pool.tile([128, N], BF16, name=f"kTh{h}", tag=f"kTh{h}")
                   for h in range(2)]
            vT2 = proj_pool.tile([128, N], BF16, name="vT2", tag="vT2")
            for t in (qTh[0], kTh[0]):
                nc.gpsimd.memset(t[64:128, :], 0.0)
            for t in (qTh[1], kTh[1]):
                nc.gpsimd.memset(t[0:64, :], 0.0)
            # v natural [j, e] in bf16, ones-augmented per head (ones column
            # FIRST so the softmax denominator lands on psum partition 0):
            # v2aug[:, jc, 0]=1, [1:65]=v_h0, [65]=1, [66:130]=v_h1
            v2aug = proj_pool.tile([128, N_JC, 130], BF16, name="v2aug",
                                   tag="v2aug")
            nc.gpsimd.memset(v2aug[:, :, 0:1], 1.0)
            nc.gpsimd.memset(v2aug[:, :, 65:66], 1.0)

            def proj_chunk(wname, i8, dsts):
                sl = slice(i8 * 512, (i8 + 1) * 512)
                ps = p1_pool.tile([128, 512], F32, name="ps", tag="p1")
                for dc in range(4):
                    nc.tensor.matmul(
                        ps[:, 0:512],
                        w_sb[wname][:, dc, :],
                        xt_sb[dc][:, sl],
                        start=(dc == 0),
                        stop=(dc == 3),
                    )
                if dsts is None:
                    nc.vector.tensor_copy(vT2[:, sl], ps[:, 0:512])
                else:
                    nc.vector.tensor_copy(dsts[0][0:64, sl], ps[0:64, 0:512])
                    nc.vector.tensor_copy(dsts[1][64:128, sl],
                                          ps[64:128, 0:512])

            def attention_jcs(ic, h, pout, jcs):
                for jc in jcs:
                    jsl = slice(jc * 128, (jc + 1) * 128)
                    sc = ps_pool.tile([128, IC], F32, name="sc", tag="ps")
                    for n2 in range(2):
                        nsl = slice(n2 * 512, (n2 + 1) * 512)
                        nc.tensor.matmul(
                            sc[:, nsl],
                            kTh[h][:, jsl],
                            qTh[h][:, ic * IC + n2 * 512:
                                   ic * IC + (n2 + 1) * 512],
                            start=True,
                            stop=True,
                        )
                    pt = pt_pool.tile([128, IC], BF16, name="pt", tag="pt")
                    nc.scalar.activation(
                        pt[:], sc[:], mybir.ActivationFunctionType.Exp,
                        scale=SCALE,
                    )
                    for n2 in range(2):
                        nsl = slice(n2 * 512, (n2 + 1) * 512)
                        nc.tensor.matmul(
                            pout[:, nsl],
                            v2aug[:, jc, h * 65:h * 65 + 65],
                            pt[:, nsl],
                            start=(jc == 0),
                            stop=(jc == N_JC - 1),
                        )

            def finish_head(h, pout, outu):
                # row 0 = denom, rows 1..64 = unnormalized out^T
                ou = norm_pool.tile([65, IC], F32, name=f"outu{h}",
                                    tag=f"outu{h}")
                nc.vector.tensor_copy(ou[:], pout[:])
                outu.append(ou)

            def attention_head(ic, h, outu):
                pout = po_pool.tile([65, IC], F32, name="pout", tag="po")
                attention_jcs(ic, h, pout, range(N_JC))
                finish_head(h, pout, outu)

            # q's first i-chunks feed the very first scores matmuls, then k/v
            # interleave chunk-wise (with v transposes) so attention can start
            # while the projection tail is still running.
            for i8 in range(2):
                proj_chunk("wq", i8, qTh)
            pout00 = None
            for i8 in range(8):
                proj_chunk("wk", i8, kTh)
                proj_chunk("wv", i8, None)
                if i8 >= 2:
                    proj_chunk("wq", i8, qTh)
                for jc in range(4 * i8, 4 * i8 + 4):
                    psb = p1_pool.tile([128, 128], BF16, name="psb", tag="p1")
                    nc.tensor.transpose(
                        psb[:, 0:128], vT2[:, jc * 128:(jc + 1) * 128],
                        ident[:],
                    )
                    nc.vector.tensor_copy(v2aug[:, jc, 1:65], psb[:, 0:64])
                    nc.vector.tensor_copy(v2aug[:, jc, 66:130],
                                          psb[:, 64:128])
                # attention (ic0, h0) starts as soon as each j-block's
                # k/v tiles exist, overlapping the projection tail
                if pout00 is None:
                    pout00 = po_pool.tile([65, IC], F32, name="pout",
                                          tag="po")
                attention_jcs(0, 0, pout00, range(4 * i8, 4 * i8 + 4))

            # ---- P2+P3: attention + normalize + output projection ----------
            norm_tiles = {}

            def norm_h(ic, h, outu):
                # Per-head normalize. The 1024 distinct denominators take a
                # DRAM round-trip: row -> dram -> [128, 8] spread -> tiny DVE
                # reciprocal -> dram -> partition-broadcast DMA load. This
                # keeps multi-us reciprocals out of the in-order DVE stream.
                if h == 0:
                    norm_tiles[ic] = (
                        norm_pool.tile([128, IC], F32, name="st1", tag="st1"),
                        norm_pool.tile([128, IC], F32, name="rec", tag="rec"),
                        norm_pool.tile([128, IC], BF16, name="outn",
                                       tag="outn"),
                    )
                st1, rec, outn = norm_tiles[ic]
                psl = slice(h * 64, (h + 1) * 64)
                idx = ic * 2 + h
                spr = norm_pool.tile([128, 8], F32, name="spr", tag="spr")
                spro = norm_pool.tile([128, 8], F32, name="spro", tag="spro")
                nc.sync.dma_start(dd1[idx:idx + 1, :], outu[h][0:1, :])
                spread_ap = bass.AP(
                    tensor=dd1.tensor, offset=idx * 1024,
                    ap=[[8, 128], [1, 8]],
                )
                nc.sync.dma_start(spr[:, :], spread_ap)
                nc.vector.reciprocal(spro[:, :], spr[:, :])
                spread_o = bass.AP(
                    tensor=dd2.tensor, offset=idx * 1024,
                    ap=[[8, 128], [1, 8]],
                )
                nc.sync.dma_start(spread_o, spro[:, :])
                bcast_ap = bass.AP(
                    tensor=dd2.tensor, offset=idx * 1024,
                    ap=[[0, 64], [1, 1024]],
                )
                nc.sync.dma_start(rec[psl, :], bcast_ap)
                nc.sync.dma_start(st1[psl, :], outu[h][1:65, :])
                nc.vector.tensor_mul(outn[psl, :], st1[psl, :], rec[psl, :])
                return outn

            def p3_proj(ic, outn):
                # partial out projection: pT[oc, i] = wo[:, oc].T @ outn[:, i]
                for oc in range(4):
                    for n2 in range(2):
                        nsl = slice(n2 * 512, (n2 + 1) * 512)
                        pp = p1_pool.tile([128, 512], F32, name="pp", tag="p1")
                        nc.tensor.matmul(
                            pp[:, 0:512],
                            wo_sb[:, oc * 128:(oc + 1) * 128],
                            outn[:, nsl],
                            start=True, stop=True,
                        )
                        st = stage_pool.tile([128, 512], F32, name="st",
                                             tag="st")
                        nc.vector.tensor_copy(st[:], pp[:, 0:512])
                        nc.sync.dma_start(
                            pT_d[oc * 128:(oc + 1) * 128,
                                 ic * IC + n2 * 512:ic * IC + (n2 + 1) * 512],
                            st[:],
                        )

            # Software-pipelined emission, chosen so the in-order PE and
            # DVE instruction streams never wait on cross-engine chains:
            #   norm(ic-1,h1) before att(ic,h0); proj(ic-1) between the two
            #   head loops of att(ic); norm(ic,h0) before att(ic,h1).
            prev = None
            for ic in range(N_IC):
                if prev is not None:
                    norm_h(prev[0], 1, prev[1])
                outu = []
                if ic == 0:
                    finish_head(0, pout00, outu)
                else:
                    attention_head(ic, 0, outu)
                if prev is not None:
                    p3_proj(prev[0], norm_tiles[prev[0]][2])
                norm_h(ic, 0, outu)
                attention_head(ic, 1, outu)
                prev = (ic, outu)
            norm_h(prev[0], 1, prev[1])
            p3_proj(prev[0], norm_tiles[prev[0]][2])
    nc.compile()
    return nc


_CACHE = {}


def _get_nc():
    if "nc" not in _CACHE:
        _CACHE["nc"] = build_kernel()
    return _CACHE["nc"]


def make_in_map(x, Wq, Wkv, Wo, core):
    bf = ml_dtypes.bfloat16
    b, p = divmod(core, 4)
    cs = slice(128 * p, 128 * (p + 1))
    return {
        "xT": np.ascontiguousarray(x[b].T).astype(bf),
        "wq": np.ascontiguousarray(Wq[:, cs]).astype(bf),
        "wk": np.ascontiguousarray(Wkv[:, :D][:, cs]).astype(bf),
        "wv": np.ascontiguousarray(Wkv[:, D:][:, cs]).astype(bf),
        "wo": np.ascontiguousarray(Wo[cs, :]).astype(bf),
    }


def kernel(x, Wq, Wkv, Wo, bo):
    x = np.asarray(x, dtype=np.float32)
    Wq = np.asarray(Wq, dtype=np.float32)
    Wkv = np.asarray(Wkv, dtype=np.float32)
    Wo = np.asarray(Wo, dtype=np.float32)
    bo = np.asarray(bo, dtype=np.float32)

    nc = _get_nc()
    in_maps = [make_in_map(x, Wq, Wkv, Wo, c) for c in range(N_CORES)]
    res = run_bass_kernel_spmd(nc, in_maps, core_ids=list(range(N_CORES)))
    out = np.empty((B, N, D), dtype=np.float32)
    for b in range(B):
        acc = res.results[4 * b]["pT"].copy()
        for p in range(1, 4):
            acc += res.results[4 * b + p]["pT"]
        out[b] = acc.T + bo
    return out

